# revision 32
# baseline (speedup 1.0000x reference)
"""GCN node classifier (2x spmm + classifier + log_softmax) on 8 trn2 cores.

Strategy: destination-node 1D sharding (per the problem's sharding hint).
Each core owns 12,500 dst nodes and the edges pointing at them; edges are
bucketed by (dst tile, source quarter), sorted and padded to 128-edge
chunks host-side.

- Phase A is node-sharded: each core computes its own 12,544-row slice of
  T1 = x@W1+b1 (bf16 rows), AllGather'ed into the full source table.
- Per-edge source rows are fetched with GPSIMD dma_gather using NARROW
  elements: the q7 firmware only requires the row STRIDE to be a 256B
  multiple, so each descriptor moves just the valid columns (128B for
  layer 1, 80B for layer 2) instead of the whole 256B row — this halves
  gather DMA time vs the stock 256B-aligned elem_size (see
  _dma_gather_narrow).
- The segment-sum is a tensor-engine matmul per 128-edge chunk against a
  scatter matrix V[e, dst_lane] = edge_val[e], built on DVE with
  (iota == ldst) * val; chunk psums accumulate over all 4 quarters.
- Wc is folded into the layer-2 table (T2C = h1 @ (W2@Wc) + b2@Wc), so
  layer-2 gathers move 40 cols and the final epilogue is just
  bias + log_softmax.
- Scheduling: layer-1 pools are allocated before phase A (disjoint
  SBUF/PSUM, no buffer-reuse anti-deps), per-tile DVE epilogue ops are
  emitted one tile late to keep the in-order DVE stream (the bottleneck
  engine) stall-free, stores are batched into single large DMAs, and deep
  msg/vp pools keep gathers and V-builds running ahead.
All accumulation is f32 (PSUM); table values are bf16.
"""

import numpy as np
import ml_dtypes

from contextlib import ExitStack


# ---------------------------------------------------------------- config ---
class Cfg:
    M = 8                 # cores
    N_NODES = 100000
    N_EDGES = 1600000
    IN_DIM = 128
    HID = 64
    NCLS = 40
    SHARD = 12500         # real dst nodes per core
    NT = 98               # dst tiles per core (128 each)
    KSEG = 5              # chunks (of 128 edges) per (tile, quarter) segment
    SLABC = 49            # chunks per gather slab
    ZBIAS = True          # b1/b2/bc all zero (checked at runtime)
    SINGLE_PACKET = False  # multi-packet gathers (single-packet hangs >~1K idxs)
    NQUEUES = 4           # spread gathers over all 4 SWDGE queues

    @property
    def PADSHARD(self):
        return self.NT * 128

    @property
    def NPAD(self):
        return self.PADSHARD * self.M

    @property
    def QROWS(self):
        return self.NPAD // 4

    @property
    def SEG(self):
        return self.KSEG * 128

    @property
    def CQ(self):
        return self.NT * self.KSEG          # chunks per quarter

    @property
    def NSLAB(self):
        assert self.CQ % self.SLABC == 0
        return self.CQ // self.SLABC        # gather slabs per quarter

    @property
    def CHUNKS(self):
        return 4 * self.CQ

    @property
    def ASLAB(self):
        # phase-A node slabs per quarter: 1792 nodes (14 x 128)
        assert self.QROWS % 1792 == 0
        return self.QROWS // 1792


CFG = Cfg()


# ------------------------------------------------------ narrow dma_gather ---
def _dma_gather_narrow(gp, out_ap, in_ap, idxs_ap, num_idxs, num_idxs_reg,
                       elem_size, elem_step, single_packet=False, queue_num=0):
    """BassGpSimd.dma_gather (non-transpose, DRAM source) without the
    elem_size_bytes % 256 == 0 restriction. The firmware (q7 dma_gather)
    only requires the per-index ROW STRIDE to be a 256B multiple; the
    element payload is arbitrary. This lets us gather just the valid
    columns of a 256B-stride table."""
    import concourse.ap_utils as ap_utils
    import concourse.mybir as mybir
    from concourse.bass import MemorySpace

    gp._assert_queue_num(queue_num)
    assert idxs_ap.dtype == mybir.dt.int16
    assert in_ap.space == MemorySpace.DRAM
    assert idxs_ap.space == MemorySpace.SBUF
    assert out_ap.space == MemorySpace.SBUF
    assert in_ap.dtype == out_ap.dtype

    assert ap_utils.ap_is_contiguous(out_ap.ap[1:])
    assert ap_utils.ap_is_contiguous(idxs_ap.ap[1:])

    assert in_ap.ap[-1][1] == out_ap.ap[-1][1] == elem_size
    assert out_ap.ap[0][1] * out_ap.ap[1][1] == -(-num_idxs // 128) * 128

    assert in_ap.ap[0][0] == elem_step
    stride_bytes = elem_step * mybir.dt.size(in_ap.dtype)
    assert stride_bytes % 256 == 0
    stride_bytes_256 = stride_bytes // 256
    assert stride_bytes_256 < 256

    _in_ap = gp.lower_ap_dma(in_ap, for_custom_bir_dma=True)
    _idxs_ap = gp.lower_ap(idxs_ap)
    _out_ap = gp.lower_ap(out_ap)
    return gp.add_instruction(
        mybir.InstDMAGatherAnt(
            name=gp.bass.get_next_instruction_name(),
            ins=[*_in_ap, _idxs_ap,
                 gp.lower_val_access(gp.to_reg(num_idxs_reg))],
            outs=[_out_ap],
            transpose=False,
            num_idxs=num_idxs,
            elem_size=elem_size,
            stride_bytes_256=stride_bytes_256,
            gen_mode=0,
            single_packet=single_packet,
            queue_num=queue_num,
            sbuf_tokens_per_rank=0,
            sbuf_free_dim_per_rank=0,
            sbuf_free_dim_pad_per_rank=0,
            sbuf_byte_offset=0,
        )
    )


# ------------------------------------------------------------- host plan ---
def _plan(cfg, edge_row, edge_col, edge_val):
    """Bucket/sort/pad edges per core. Returns per-core arrays:
    idx16 [128, 4*CQ*128/16] int16, ldstT [128, CHUNKS] bf16, valT [128, CHUNKS] bf16.
    """
    M, SHARD, PADSHARD = cfg.M, cfg.SHARD, cfg.PADSHARD
    NT, KSEG, SEG, CQ, QROWS = cfg.NT, cfg.KSEG, cfg.SEG, cfg.CQ, cfg.QROWS

    # padded (table) node id and quarter decomposition of sources
    psrc = (edge_col // SHARD) * PADSHARD + (edge_col % SHARD)
    q_of = psrc // QROWS
    i_of = psrc % QROWS
    core_of = edge_row // SHARD
    dloc = edge_row % SHARD
    t_of = dloc // 128
    l_of = dloc % 128

    L = 4 * CQ * 128
    idx_all, ldst_all, val_all = [], [], []
    for c in range(M):
        sel = core_of == c
        # order: (quarter, tile) segment id
        segid = q_of[sel] * NT + t_of[sel]
        order = np.argsort(segid, kind="stable")
        sid = segid[order]
        idx_s = i_of[sel][order]
        l_s = l_of[sel][order]
        v_s = edge_val[sel][order]

        counts = np.bincount(sid, minlength=4 * NT)
        if counts.max() > SEG:
            raise ValueError(f"segment overflow: {counts.max()} > {SEG}")
        # place into padded stream: segment s at offset s*SEG
        starts = np.arange(4 * NT) * SEG
        pos = starts[sid] + (np.arange(sid.size) - np.concatenate(([0], np.cumsum(counts)))[sid])

        idx = np.zeros(L, dtype=np.int16)
        ldst = np.zeros(L, dtype=np.float32)
        val = np.zeros(L, dtype=np.float32)
        idx[pos] = idx_s.astype(np.int16)
        ldst[pos] = l_s.astype(np.float32)
        val[pos] = v_s.astype(np.float32)

        # wrap indices: idx i -> [i%16, i//16], replicated on all 8 q7 cores
        idxw = np.tile(idx.reshape(-1, 16).T, (8, 1)).copy()          # [128, L/16]
        ldstT = np.ascontiguousarray(ldst.reshape(-1, 128).T)         # [128, CHUNKS]
        valT = np.ascontiguousarray(val.reshape(-1, 128).T)
        idx_all.append(idxw)
        ldst_all.append(ldstT)
        val_all.append(valT)
    return idx_all, ldst_all, val_all


def _pack_x(cfg, x):
    """x [N, IN] -> padded transposed [IN, NPAD] bf16."""
    xp = np.zeros((cfg.NPAD, cfg.IN_DIM), dtype=np.float32)
    xp.reshape(cfg.M, cfg.PADSHARD, cfg.IN_DIM)[:, : cfg.SHARD] = x.reshape(
        cfg.M, cfg.SHARD, cfg.IN_DIM
    )
    return np.ascontiguousarray(xp.T).astype(ml_dtypes.bfloat16)


# --------------------------------------------------------- device program ---
def _build(cfg, timing=False):
    from concourse import bacc, tile
    import concourse.mybir as mybir

    f32 = mybir.dt.float32
    bf16 = mybir.dt.bfloat16
    i16 = mybir.dt.int16
    AOP = mybir.AluOpType
    ACT = mybir.ActivationFunctionType

    nc = bacc.Bacc("TRN2", target_bir_lowering=False, debug=False,
                   num_devices=1 if timing else cfg.M,
                   dynamic_dma_scratch_size=16384,
                   num_swdge_queues=cfg.NQUEUES)

    NPAD, QROWS, NT, KSEG, CQ, SLABC, NSLAB = (
        cfg.NPAD, cfg.QROWS, cfg.NT, cfg.KSEG, cfg.CQ, cfg.SLABC, cfg.NSLAB)
    CHUNKS, HID, NCLS, IN_DIM = cfg.CHUNKS, cfg.HID, cfg.NCLS, cfg.IN_DIM
    LQ16 = CQ * 128 // 16              # idx columns per quarter
    SLAB16 = SLABC * 128 // 16         # idx columns per slab
    NA = cfg.ASLAB                     # phase-A slabs (2048 nodes each)

    # -------- I/O
    XT = nc.dram_tensor("xt", [IN_DIM, cfg.PADSHARD], bf16, kind="ExternalInput")
    IDX = nc.dram_tensor("idx", [128, 4 * LQ16], i16, kind="ExternalInput")
    LDST = nc.dram_tensor("ldst", [128, CHUNKS], f32, kind="ExternalInput")
    VAL = nc.dram_tensor("val", [128, CHUNKS], f32, kind="ExternalInput")
    W1 = nc.dram_tensor("w1", [IN_DIM, HID], bf16, kind="ExternalInput")
    W2C = nc.dram_tensor("w2c", [HID, NCLS], bf16, kind="ExternalInput")
    B1 = nc.dram_tensor("b1", [128, 8, HID], f32, kind="ExternalInput")  # repl
    B2C = nc.dram_tensor("b2c", [128, NCLS], f32, kind="ExternalInput")
    BC = nc.dram_tensor("bc", [128, NCLS], f32, kind="ExternalInput")
    IOTA = nc.dram_tensor("iota", [128, 128], bf16, kind="ExternalInput")
    IDENT = nc.dram_tensor("ident", [128, 128], bf16, kind="ExternalInput")
    # partition-major: OUT[p, t, c] = logits[node t*128+p, c]; host unscrambles
    OUT = nc.dram_tensor("out", [128, NT * NCLS], f32, kind="ExternalOutput")

    # -------- internal DRAM (bf16 rows, 256B stride; tail cols junk)
    # phase A computes only this core's node shard; the full source table is
    # assembled with an AllGather (concat over cores = node order).
    T1S = nc.dram_tensor("t1shard", [cfg.PADSHARD, 128], bf16)
    T1F = nc.dram_tensor("t1full", [NPAD, 128], bf16, addr_space="Shared")
    T2S = nc.dram_tensor("t2shard", [cfg.PADSHARD, 128], bf16)
    T2F = nc.dram_tensor("t2full", [NPAD, 128], bf16, addr_space="Shared")

    with tile.TileContext(nc) as tc, ExitStack() as top:
        cpool = top.enter_context(tc.tile_pool(name="consts", bufs=1))
        edg = top.enter_context(tc.tile_pool(name="edg", bufs=1))
        # load order: phase-A needs w1s+XT; V-builds need iot+ldst+val;
        # gathers need idx. Epilogue consts last.
        w1s = cpool.tile([IN_DIM, HID], bf16)
        nc.sync.dma_start(out=w1s, in_=W1[:, :])
        iot = cpool.tile([128, 128], bf16)
        nc.sync.dma_start(out=iot, in_=IOTA[:, :])
        ldsts = edg.tile([128, CHUNKS], f32)
        nc.sync.dma_start(out=ldsts, in_=LDST[:, :])
        vals = edg.tile([128, CHUNKS], f32)
        nc.sync.dma_start(out=vals, in_=VAL[:, :])
        its = []
        for q in range(4):
            it = edg.tile([128, LQ16], i16, tag=f"idx{q}")
            nc.sync.dma_start(out=it, in_=IDX[:, q * LQ16:(q + 1) * LQ16])
            its.append(it)
        w2cs = cpool.tile([HID, NCLS], bf16)
        nc.sync.dma_start(out=w2cs, in_=W2C[:, :])
        b18s = cpool.tile([128, 8, HID], f32)
        nc.sync.dma_start(out=b18s, in_=B1[:, :, :])
        b2cs = cpool.tile([128, NCLS], f32)
        nc.sync.dma_start(out=b2cs, in_=B2C[:, :])
        bcs = cpool.tile([128, NCLS], f32)
        nc.sync.dma_start(out=bcs, in_=BC[:, :])
        idn = cpool.tile([128, 128], bf16)
        nc.sync.dma_start(out=idn, in_=IDENT[:, :])

        accp = top.enter_context(tc.tile_pool(name="acc", bufs=1))

        # L1 pools are allocated BEFORE phase A so their SBUF/PSUM space is
        # disjoint from phase-A pools — otherwise buffer-reuse anti-deps
        # would stall the first layer-1 gathers until all of phase A ends.
        l1s = top.enter_context(ExitStack())
        msg = l1s.enter_context(tc.tile_pool(name="msg", bufs=12))
        vp = l1s.enter_context(tc.tile_pool(name="vp", bufs=64))
        psb = l1s.enter_context(tc.tile_pool(name="psb", bufs=3, space="PSUM"))
        tp1 = l1s.enter_context(tc.tile_pool(name="tc1", bufs=3))
        tp2 = l1s.enter_context(tc.tile_pool(name="tc2", bufs=3))
        pst = l1s.enter_context(tc.tile_pool(name="pst", bufs=2, space="PSUM"))
        psc = l1s.enter_context(tc.tile_pool(name="psc", bufs=2, space="PSUM"))

        # ================= phase A: T1S = x_shard @ W1 + b1 (bf16 rows)
        # In the timing build (no collective modeled) phase A writes straight
        # into T1F so the layer-1 gathers keep their ordering dependency.
        T1DST = T1F if timing else T1S
        with tc.tile_pool(name="xa", bufs=3) as xa, \
             tc.tile_pool(name="sta", bufs=3) as sta, \
             tc.tile_pool(name="psa", bufs=1, space="PSUM") as psa:
            for s in range(cfg.PADSHARD // 1792):
                c0 = s * 1792
                xs = xa.tile([128, 1792], bf16)
                nc.sync.dma_start(out=xs, in_=XT[:, c0:c0 + 1792])
                st = sta.tile([128, 14, HID], bf16)
                for h in range(2):
                    pb = psa.tile([128, 7, HID], f32)
                    if not cfg.ZBIAS:
                        # psum pre-load with bias (ACT); matmuls accumulate
                        nc.scalar.activation(pb, b18s[:, 0:7, :], ACT.Copy)
                    for k7 in range(7):
                        k = h * 7 + k7
                        nc.tensor.matmul(pb[:, k7, :],
                                         lhsT=xs[:, k * 128:(k + 1) * 128],
                                         rhs=w1s, start=cfg.ZBIAS,
                                         stop=True)
                    nc.scalar.activation(st[:, h * 7:(h + 1) * 7, :], pb,
                                         ACT.Copy)
                dst = T1DST[c0:c0 + 1792, 0:HID].rearrange(
                    "(k p) f -> p k f", p=128)
                nc.sync.dma_start(out=dst, in_=st)
        if not timing:
            nc.gpsimd.collective_compute(
                "AllGather", mybir.AluOpType.bypass,
                replica_groups=[list(range(cfg.M))],
                ins=[T1S[:, :]], outs=[T1F[:, :]])

        # ============ spmm layer runner: per-tile single psum group across
        # all 4 quarters (slabs for all quarters retire in lockstep), with a
        # fused per-tile epilogue. Gathers fetch only `width` valid columns.
        def spmm_layer(quarter_tab, width, epilogue, pools):
            msg, vp, psb = pools
            slabs = [[None] * NSLAB for _ in range(4)]

            def ensure_slab(q, s):
                if slabs[q][s] is None:
                    mt = msg.tile([128, SLABC, width], bf16)
                    _dma_gather_narrow(
                        nc.gpsimd, mt, quarter_tab(q)[:, 0:width],
                        its[q][:, s * SLAB16:(s + 1) * SLAB16],
                        num_idxs=SLABC * 128, num_idxs_reg=SLABC * 128,
                        elem_size=width, elem_step=128,
                        single_packet=cfg.SINGLE_PACKET,
                        queue_num=(q * NSLAB + s) % cfg.NQUEUES)
                    slabs[q][s] = mt
                return slabs[q][s]

            for t in range(NT):
                ps = psb.tile([128, width], f32)
                for q in range(4):
                    for k in range(KSEG):
                        j = t * KSEG + k                 # chunk in quarter
                        gj = q * CQ + j                  # global chunk
                        v = vp.tile([128, 128], bf16)
                        nc.vector.tensor_scalar(
                            v, iot, ldsts[:, gj:gj + 1], vals[:, gj:gj + 1],
                            AOP.is_equal, AOP.mult)
                        mt = ensure_slab(q, j // SLABC)
                        nc.tensor.matmul(ps, lhsT=v,
                                         rhs=mt[:, j % SLABC, :],
                                         start=(q == 0 and k == 0),
                                         stop=(q == 3 and k == KSEG - 1))
                epilogue(t, ps)

        # ================= layer 1 + fused epilogue: T2S = relu(h1)@W2C+b2C
        if True:
            t2acc = accp.tile([128, NT, NCLS], bf16, tag="t2acc")
            pend1 = []

            def epi1_flush():
                tt, pps2 = pend1.pop(0)
                nc.vector.tensor_tensor(t2acc[:, tt, :], pps2, b2cs, AOP.add)

            def epi1(t, ps):
                h1r = tp1.tile([128, HID], bf16)
                nc.scalar.activation(h1r, ps, ACT.Relu)
                ptr = pst.tile([HID, 128], bf16)
                nc.tensor.transpose(ptr, h1r, idn)
                h1t = tp2.tile([HID, 128], bf16)
                nc.scalar.activation(h1t, ptr, ACT.Copy)
                ps2 = psc.tile([128, NCLS], f32)
                nc.tensor.matmul(ps2, lhsT=h1t, rhs=w2cs, start=True,
                                 stop=True)
                pend1.append((t, ps2))
                if len(pend1) > 1:
                    epi1_flush()

            spmm_layer(lambda q: T1F[q * QROWS:(q + 1) * QROWS, :], HID,
                       epi1, (msg, vp, psb))
            while pend1:
                epi1_flush()
            # one batched store of the whole shard table (avoids 98 small
            # DMAs' fixed per-copy overheads)
            t2dst = T2S[:, 0:NCLS].rearrange("(t p) c -> p t c", p=128)
            nc.sync.dma_start(out=t2dst, in_=t2acc)
            if not timing:
                nc.gpsimd.collective_compute(
                    "AllGather", mybir.AluOpType.bypass,
                    replica_groups=[list(range(cfg.M))],
                    ins=[T2S[:, :]], outs=[T2F[:, :]])
            l1s.close()   # release L1 SBUF/PSUM pools before layer 2

        # ================= layer 2 + fused epilogue: logits + log_softmax
        with tc.tile_pool(name="msg2", bufs=12) as msg2, \
             tc.tile_pool(name="vp2", bufs=64) as vp2, \
             tc.tile_pool(name="psb2", bufs=3, space="PSUM") as psb2, \
             tc.tile_pool(name="te1", bufs=3) as te1:
            lgacc = accp.tile([128, NT, NCLS], f32, tag="lgacc")
            negmacc = accp.tile([128, NT], f32, tag="negmacc")
            smacc = accp.tile([128, NT], f32, tag="smacc")

            def epi2(t, ps):
                nc.vector.tensor_tensor(lgacc[:, t, :], ps, bcs, AOP.add)
                nc.vector.tensor_reduce(negmacc[:, t:t + 1], lgacc[:, t, :],
                                        mybir.AxisListType.X, AOP.max,
                                        negate=True)
                et = te1.tile([128, NCLS], f32, tag="et")
                nc.scalar.activation(et, lgacc[:, t, :], ACT.Exp,
                                     bias=negmacc[:, t:t + 1],
                                     accum_out=smacc[:, t:t + 1])

            spmm_layer(lambda q: T2F[q * QROWS:(q + 1) * QROWS, :], NCLS,
                       epi2, (msg2, vp2, psb2))

            # one Ln over all tiles; -(max + ln(sumexp)) applied as bias
            lnacc = accp.tile([128, NT], f32, tag="lnacc")
            nc.scalar.activation(lnacc, smacc, ACT.Ln)
            negsh = accp.tile([128, NT], f32, tag="negsh")
            nc.vector.tensor_tensor(negsh, negmacc, lnacc, AOP.subtract)
            otacc = accp.tile([128, NT, NCLS], f32, tag="otacc")
            for t in range(NT):
                nc.scalar.activation(otacc[:, t, :], lgacc[:, t, :],
                                     ACT.Identity, bias=negsh[:, t:t + 1])
            nc.sync.dma_start(out=OUT[:, :],
                              in_=otacc.rearrange("p t c -> p (t c)"))

    nc.compile()
    return nc


_NC_CACHE = {}


def _get_nc(cfg):
    key = (cfg.KSEG, cfg.SLABC, cfg.ZBIAS)
    if key not in _NC_CACHE:
        _NC_CACHE[key] = _build(cfg)
    return _NC_CACHE[key]


# ------------------------------------------------------------------ main ---
def kernel(x, edge_row, edge_col, edge_val, W1, b1, W2, b2, Wc, bc,
           _run_kwargs=None):
    from concourse.bass_utils import run_bass_kernel_spmd

    cfg = CFG
    x = np.asarray(x, dtype=np.float32)
    edge_row = np.asarray(edge_row, dtype=np.int64)
    edge_col = np.asarray(edge_col, dtype=np.int64)
    edge_val = np.asarray(edge_val, dtype=np.float32)
    W1 = np.asarray(W1, dtype=np.float32)
    W2 = np.asarray(W2, dtype=np.float32)
    Wc = np.asarray(Wc, dtype=np.float32)
    b1 = np.asarray(b1, dtype=np.float32)
    b2 = np.asarray(b2, dtype=np.float32)
    bc = np.asarray(bc, dtype=np.float32)

    cfg.ZBIAS = not (b1.any() or b2.any() or bc.any())
    while True:
        try:
            idx_all, ldst_all, val_all = _plan(cfg, edge_row, edge_col, edge_val)
            break
        except ValueError:
            cfg.KSEG += 1
            if cfg.KSEG > 40:
                raise

    xT = _pack_x(cfg, x)
    w1h = W1.astype(ml_dtypes.bfloat16)
    W2C = (W2 @ Wc).astype(ml_dtypes.bfloat16)          # folded classifier
    b2c = (b2 @ Wc).astype(np.float32)
    iota = np.tile(np.arange(128, dtype=np.float32), (128, 1)).astype(
        ml_dtypes.bfloat16)
    ident = np.eye(128, dtype=np.float32).astype(ml_dtypes.bfloat16)
    b1r = np.tile(b1, (128, 8, 1)).reshape(128, 8, cfg.HID).astype(np.float32)
    b2cr = np.tile(b2c, (128, 1)).astype(np.float32)
    bcr = np.tile(bc, (128, 1)).astype(np.float32)

    nc = _get_nc(cfg)
    in_maps = []
    for c in range(cfg.M):
        in_maps.append({
            "xt": np.ascontiguousarray(
                xT[:, c * cfg.PADSHARD:(c + 1) * cfg.PADSHARD]),
            "idx": idx_all[c], "ldst": ldst_all[c],
            "val": val_all[c], "w1": w1h, "w2c": W2C,
            "b1": b1r, "b2c": b2cr, "bc": bcr, "iota": iota, "ident": ident,
        })
    kw = dict(_run_kwargs or {})
    res = run_bass_kernel_spmd(nc, in_maps, core_ids=list(range(cfg.M)), **kw)
    outs = []
    for c in range(cfg.M):
        o = res.results[c]["out"].reshape(128, cfg.NT, cfg.NCLS)
        outs.append(o.transpose(1, 0, 2).reshape(cfg.PADSHARD, cfg.NCLS)[: cfg.SHARD])
    out = np.concatenate(outs, axis=0)
    kernel.last_results = res
    return out.astype(np.float32)
